# revision 11
# baseline (speedup 1.0000x reference)
"""DistSAGE 3-layer GraphSAGE forward on 8 TRN2 NeuronCores (Bass/Tile).

Strategy (graph/data parallel, per the DistSAGE recipe):
  - Partition the 512 seed nodes across 8 cores (64 each); build per-core
    dependency-driven blocks on the host (standard DGL block construction,
    pure index math): layer-2 dsts = seeds, layer-1 dsts = seeds + their
    layer-2 sources, layer-0 dsts = layer-1 dsts + their layer-1 sources.
    No inter-core communication needed; weights replicated.
  - Row-shard the feature table: each core gets a compact table with only
    the x rows its block touches, laid out in per-dst-tile "bands" so each
    dma_gather call addresses rows with int16 indices relative to a
    32768-row window (the custom gather ucode's index width).
  - Per 128-dst tile on device: dma_gather edge messages (128 edges/chunk,
    1024/call), build S'[e,d] = (edst[e]==d)*invdeg[e] with one fused DVE
    tensor_scalar against a resident iota, accumulate
    meanT[f,d] += msgs.T @ S' on the TensorEngine in PSUM.  A per-tile
    "self chunk" against an identity rhs yields h_dstT the same way
    (doubling as the transpose).  Then Y[d,:] = meanT.T@W_neigh +
    h_dstT.T@W_self (4 PSUM-accumulated matmuls), + bias, ReLU, DMA the
    tile to DRAM for the next layer's gather.
"""

import numpy as np

P = 128
NCORES = 8
NUM_DST = (61952, 5632, 512)
N_SRC = (681472, 61952, 5632)
FEAT = 256  # feature width entering every layer
OUTW = (256, 256, 19)
SEEDS_PER_CORE = NUM_DST[2] // NCORES  # 64
WINDOW = 32768
NI = 1024  # indices per dma_gather call (ring-safe max)
CPC = NI // P  # chunks per call = 8

USE_INDIRECT = False  # fallback: indirect_dma_start (int32), ~25% slower


# ---------------------------------------------------------------------------
# Host-side block construction
# ---------------------------------------------------------------------------


def _block_for_core(c, esrc0, edst0, esrc1, edst1, esrc2, edst2, deg0, deg1, deg2):
    seeds = np.arange(c * SEEDS_PER_CORE, (c + 1) * SEEDS_PER_CORE, dtype=np.int64)

    # layer 2: seeds are the dsts
    sel2 = (edst2 >= seeds[0]) & (edst2 < seeds[0] + SEEDS_PER_CORE)
    es2, ed2g = esrc2[sel2], edst2[sel2]
    l1_out = np.concatenate([seeds, np.setdiff1d(np.unique(es2), seeds)])
    n1 = len(l1_out)

    # layer 1: dsts = l1_out
    pos1 = np.full(NUM_DST[1], -1, np.int32)
    pos1[l1_out] = np.arange(n1, dtype=np.int32)
    sel1 = pos1[edst1] >= 0
    es1, ed1g = esrc1[sel1], edst1[sel1]
    ed1 = pos1[ed1g].astype(np.int64)
    inv1 = (1.0 / np.maximum(deg1[ed1g], 1.0)).astype(np.float32)
    l0_out = np.concatenate([l1_out, np.setdiff1d(np.unique(es1), l1_out)])
    n0 = len(l0_out)

    # layer 0: dsts = l0_out
    pos0 = np.full(NUM_DST[0], -1, np.int32)
    pos0[l0_out] = np.arange(n0, dtype=np.int32)
    sel0 = pos0[edst0] >= 0
    es0, ed0g = esrc0[sel0], edst0[sel0]
    ed0 = pos0[ed0g].astype(np.int64)
    inv0 = (1.0 / np.maximum(deg0[ed0g], 1.0)).astype(np.float32)

    ed2 = pos1[ed2g].astype(np.int64)  # seeds occupy slots 0..63
    inv2 = (1.0 / np.maximum(deg2[ed2g], 1.0)).astype(np.float32)
    es2l = pos1[es2].astype(np.int64)  # h2 local rows
    es1l = pos0[es1].astype(np.int64)  # h1 local rows

    return dict(
        l1_out=l1_out, l0_out=l0_out, n1=n1, n0=n0,
        e0=(es0.astype(np.int64), ed0, inv0),
        e1=(es1l, ed1, inv1),
        e2=(es2l, ed2, inv2),
    )


def _group_edges_by_tile(es, ed, inv, n_tiles):
    """Sort edges by (dst tile, src); return per-tile (src, dst%128, inv)."""
    tile = ed // P
    order = np.lexsort((es, tile))
    es, ed, inv, tile = es[order], ed[order], inv[order], tile[order]
    starts = np.searchsorted(tile, np.arange(n_tiles))
    ends = np.searchsorted(tile, np.arange(n_tiles) + 1)
    return [
        (es[s:e], ed[s:e] - t * P, inv[s:e])
        for t, (s, e) in enumerate(zip(starts, ends))
    ]


class LayerPlan:
    def __init__(self, n_tiles, chunks_per_tile):
        self.n_tiles = n_tiles
        self.chunks_per_tile = chunks_per_tile
        self.col_self = []
        self.col_edges = []
        col = 0
        for t in range(n_tiles):
            self.col_self.append(col)
            col += 1
            self.col_edges.append(list(range(col, col + chunks_per_tile[t])))
            col += chunks_per_tile[t]
        self.n_cols = col
        self.n_calls = -(-col // CPC)
        self.n_cols_pad = self.n_calls * CPC
        self.gidx = []  # [NCORES][128, n_cols_pad] int64 table rows
        self.edst = []  # [NCORES][128, n_cols_pad] f32, -1 = padding
        self.invd = []  # [NCORES][128, n_cols_pad] f32
        self.call_base = []  # [NCORES][n_calls] int64


def _plan_layer(per_core_tiles, n_tiles):
    chunks = [
        max(1, max(-(-len(per_core_tiles[c][t][0]) // P) for c in range(NCORES)))
        for t in range(n_tiles)
    ]
    return LayerPlan(n_tiles, chunks)


def _fill_plan(plan, per_core_tiles, self_rows, pad_row):
    """self_rows[c][t]: int64[128] table rows for the tile's h_dst.
    pad_row[c][t]: row used for padded slots (must lie in the tile's band)."""
    for c in range(NCORES):
        gidx = np.zeros((P, plan.n_cols_pad), np.int64)
        edst = np.full((P, plan.n_cols_pad), -1.0, np.float32)
        invd = np.zeros((P, plan.n_cols_pad), np.float32)
        for t in range(plan.n_tiles):
            gidx[:, plan.col_self[t]] = self_rows[c][t]
            es, ed, inv = per_core_tiles[c][t]
            n = len(es)
            cols = plan.col_edges[t]
            bi = np.full(len(cols) * P, pad_row[c][t], np.int64)
            bd = np.full(len(cols) * P, -1.0, np.float32)
            bv = np.zeros(len(cols) * P, np.float32)
            bi[:n], bd[:n], bv[:n] = es, ed, inv
            for j, col in enumerate(cols):
                gidx[:, col] = bi[j * P : (j + 1) * P]
                edst[:, col] = bd[j * P : (j + 1) * P]
                invd[:, col] = bv[j * P : (j + 1) * P]
        # whole padded columns at the stream tail keep gidx=last pad_row
        if plan.n_cols_pad > plan.n_cols:
            gidx[:, plan.n_cols :] = pad_row[c][plan.n_tiles - 1]
        plan.gidx.append(gidx)
        plan.edst.append(edst)
        plan.invd.append(invd)


def build_host(inputs):
    esrc0 = np.asarray(inputs["esrc0"]).astype(np.int64)
    edst0 = np.asarray(inputs["edst0"]).astype(np.int64)
    esrc1 = np.asarray(inputs["esrc1"]).astype(np.int64)
    edst1 = np.asarray(inputs["edst1"]).astype(np.int64)
    esrc2 = np.asarray(inputs["esrc2"]).astype(np.int64)
    edst2 = np.asarray(inputs["edst2"]).astype(np.int64)
    x = np.asarray(inputs["x"], dtype=np.float32)

    deg0 = np.bincount(edst0, minlength=NUM_DST[0]).astype(np.float32)
    deg1 = np.bincount(edst1, minlength=NUM_DST[1]).astype(np.float32)
    deg2 = np.bincount(edst2, minlength=NUM_DST[2]).astype(np.float32)

    blocks = [
        _block_for_core(c, esrc0, edst0, esrc1, edst1, esrc2, edst2, deg0, deg1, deg2)
        for c in range(NCORES)
    ]

    n0_pad = max(-(-b["n0"] // P) for b in blocks) * P
    n1_pad = max(-(-b["n1"] // P) for b in blocks) * P
    T0, T1, T2 = n0_pad // P, n1_pad // P, 1

    tiles0 = [_group_edges_by_tile(*b["e0"], T0) for b in blocks]
    tiles1 = [_group_edges_by_tile(*b["e1"], T1) for b in blocks]
    tiles2 = [_group_edges_by_tile(*b["e2"], T2) for b in blocks]

    plan0 = _plan_layer(tiles0, T0)
    plan1 = _plan_layer(tiles1, T1)
    plan2 = _plan_layer(tiles2, T2)

    # layer-0 self rows = the tiles' global dst ids (padded slots -> row 0 of
    # x; they're masked downstream because no edge/out references them)
    l0_padded = []
    for b in blocks:
        v = np.zeros(T0 * P, np.int64)
        v[: b["n0"]] = b["l0_out"]
        l0_padded.append(v)
    self0 = [[l0_padded[c][t * P : (t + 1) * P] for t in range(T0)] for c in range(NCORES)]
    pad0 = [[l0_padded[c][t * P] for t in range(T0)] for c in range(NCORES)]
    _fill_plan(plan0, tiles0, self0, pad0)

    ar = np.arange(P, dtype=np.int64)
    selfL = lambda T: [[t * P + ar for t in range(T)] for _ in range(NCORES)]
    padL = lambda T: [[t * P for t in range(T)] for _ in range(NCORES)]
    _fill_plan(plan1, tiles1, selfL(T1), padL(T1))
    _fill_plan(plan2, tiles2, selfL(T2), padL(T2))

    # ---- layer 0: banded compact x table per core + index remap ----
    # Band t holds the x rows tile t references.  Band OFFSETS must be
    # uniform across cores (the window base is baked into the SPMD-shared
    # graph), so pad each band to the max size over cores.
    bands = []  # [NCORES][T0] sorted int64 row arrays
    for c in range(NCORES):
        bands.append(
            [
                np.unique(
                    plan0.gidx[c][:, [plan0.col_self[t]] + plan0.col_edges[t]]
                )
                for t in range(T0)
            ]
        )
    band_size = np.array(
        [max(len(bands[c][t]) for c in range(NCORES)) for t in range(T0)], np.int64
    )
    band_start = np.concatenate([[0], np.cumsum(band_size)])  # [T0+1], uniform

    for c in range(NCORES):
        gidx, new = plan0.gidx[c], np.zeros_like(plan0.gidx[c])
        for t in range(T0):
            lo = band_start[t]
            band = bands[c][t]
            cols = [plan0.col_self[t]] + plan0.col_edges[t]
            sl = gidx[:, cols]
            loc = np.searchsorted(band, sl)
            assert (band[loc] == sl).all()
            new[:, cols] = lo + loc
        if plan0.n_cols_pad > plan0.n_cols:
            band = bands[c][T0 - 1]
            sl = gidx[:, plan0.n_cols :]
            new[:, plan0.n_cols :] = band_start[T0 - 1] + np.searchsorted(band, sl)
        plan0.gidx[c] = new

    # per-call window bases (uniform); layers 1/2 fit int16 with base 0
    col_tile = np.zeros(plan0.n_cols_pad, np.int64)
    for t in range(T0):
        for col in [plan0.col_self[t]] + plan0.col_edges[t]:
            col_tile[col] = t
    col_tile[plan0.n_cols :] = T0 - 1
    bases = np.array(
        [band_start[col_tile[k * CPC]] for k in range(plan0.n_calls)], np.int64
    )
    for c in range(NCORES):
        for k in range(plan0.n_calls):
            sl = plan0.gidx[c][:, k * CPC : (k + 1) * CPC]
            assert sl.min() >= bases[k] and sl.max() < bases[k] + WINDOW, (
                c, k, int(bases[k]), int(sl.min()), int(sl.max()),
            )
        plan0.call_base.append(bases)
    for plan in (plan1, plan2):
        plan.call_base = [np.zeros(plan.n_calls, np.int64) for _ in range(NCORES)]

    xc_len_pad = -(-int(band_start[T0]) // P) * P
    xcs = []
    for c in range(NCORES):
        t = np.zeros((xc_len_pad, FEAT), np.float32)
        for tt in range(T0):
            b = bands[c][tt]
            t[band_start[tt] : band_start[tt] + len(b)] = x[b]
        xcs.append(t)

    return dict(
        plans=(plan0, plan1, plan2),
        T=(T0, T1, T2),
        n0_pad=n0_pad,
        n1_pad=n1_pad,
        xc_len_pad=xc_len_pad,
        xcs=xcs,
        blocks=blocks,
        weights=tuple(
            (
                np.asarray(inputs[f"W_self{l}"], np.float32),
                np.asarray(inputs[f"W_neigh{l}"], np.float32),
                np.asarray(inputs[f"b{l}"], np.float32),
            )
            for l in range(3)
        ),
    )


# ---------------------------------------------------------------------------
# Numpy simulation of the device kernel (validation aid)
# ---------------------------------------------------------------------------


def simulate_core(meta, c):
    plans = meta["plans"]
    weights = meta["weights"]
    table = meta["xcs"][c]
    for l, plan in enumerate(plans):
        ws, wn, b = weights[l]
        out = np.zeros((plan.n_tiles * P, OUTW[l]), np.float32)
        for t in range(plan.n_tiles):
            hd = table[plan.gidx[c][:, plan.col_self[t]]]
            aggT = np.zeros((FEAT, P), np.float32)
            for col in plan.col_edges[t]:
                msgs = table[plan.gidx[c][:, col]]
                ed = plan.edst[c][:, col]
                iv = plan.invd[c][:, col]
                S = (ed[:, None] == np.arange(P)[None, :]) * iv[:, None]
                aggT += msgs.T.astype(np.float32) @ S.astype(np.float32)
            y = hd @ ws + aggT.T @ wn + b
            if l < 2:
                y = np.maximum(y, 0.0)
            out[t * P : (t + 1) * P] = y
        table = out
    return table[:SEEDS_PER_CORE]


# ---------------------------------------------------------------------------
# Device kernel
# ---------------------------------------------------------------------------


def _wrap_idx16(gidx_rel):
    """[128, n_cols_pad] relative rows -> dma_gather idx layout
    [128, n_calls*NI/16] int16 (16-partition wrap, replicated x8)."""
    n_calls = gidx_rel.shape[1] // CPC
    out = np.zeros((P, n_calls * NI // 16), np.int16)
    for k in range(n_calls):
        flat = gidx_rel[:, k * CPC : (k + 1) * CPC].T.reshape(-1)  # i = j*128+p
        w = flat.reshape(NI // 16, 16).T.astype(np.int16)  # [16, NI/16]
        out[:16, k * (NI // 16) : (k + 1) * (NI // 16)] = w
    for rep in range(1, 8):
        out[rep * 16 : (rep + 1) * 16] = out[:16]
    return out


def run_device(meta, trace=False):
    import concourse.bacc as bacc
    import concourse.bass as bass
    import concourse.tile as tile
    import concourse.mybir as mybir
    from concourse.bass_utils import run_bass_kernel_spmd

    plans = meta["plans"]
    T0, T1, T2 = meta["T"]
    f32 = mybir.dt.float32

    nc = bacc.Bacc("TRN2", target_bir_lowering=False, debug=False, num_devices=NCORES)

    xc = nc.dram_tensor("xc", [meta["xc_len_pad"], FEAT], f32, kind="ExternalInput")
    iota_d = nc.dram_tensor("iota", [P, P], f32, kind="ExternalInput")
    ident_d = nc.dram_tensor("ident", [P, P], f32, kind="ExternalInput")
    h1buf = nc.dram_tensor("h1buf", [meta["n0_pad"], FEAT], f32)
    h2buf = nc.dram_tensor("h2buf", [meta["n1_pad"], FEAT], f32)
    out_d = nc.dram_tensor("out", [SEEDS_PER_CORE, OUTW[2]], f32, kind="ExternalOutput")

    idx_d, edst_d, invd_d, w_d = [], [], [], []
    for l, plan in enumerate(plans):
        if USE_INDIRECT:
            idx_d.append(
                nc.dram_tensor(f"gidx{l}", [P, plan.n_cols_pad], mybir.dt.int32,
                               kind="ExternalInput")
            )
        else:
            idx_d.append(
                nc.dram_tensor(f"gidx{l}", [P, plan.n_calls * NI // 16],
                               mybir.dt.int16, kind="ExternalInput")
            )
        edst_d.append(
            nc.dram_tensor(f"edst{l}", [P, plan.n_cols_pad], f32, kind="ExternalInput")
        )
        invd_d.append(
            nc.dram_tensor(f"invd{l}", [P, plan.n_cols_pad], f32, kind="ExternalInput")
        )
        w_d.append(
            (
                nc.dram_tensor(f"ws{l}", [FEAT, OUTW[l]], f32, kind="ExternalInput"),
                nc.dram_tensor(f"wn{l}", [FEAT, OUTW[l]], f32, kind="ExternalInput"),
                nc.dram_tensor(f"bias{l}", [P, OUTW[l]], f32, kind="ExternalInput"),
            )
        )

    tables = [xc, h1buf, h2buf]
    outs = [h1buf, h2buf, out_d]

    with tile.TileContext(nc) as tc:
        with (
            tc.tile_pool(name="const", bufs=1) as cpool,
            tc.tile_pool(name="msgs", bufs=6 if USE_INDIRECT else 3) as mpool,
            tc.tile_pool(name="sel", bufs=4) as spool,
            tc.tile_pool(name="acc", bufs=2) as apool,
            tc.tile_pool(name="outp", bufs=3) as opool,
            tc.tile_pool(name="pagg", bufs=1, space="PSUM") as pa,
            tc.tile_pool(name="py", bufs=2, space="PSUM") as pypool,
        ):
            iota_t = cpool.tile([P, P], f32, tag="iota")
            nc.sync.dma_start(out=iota_t[:], in_=iota_d[:])
            ident_t = cpool.tile([P, P], f32, tag="ident")
            nc.sync.dma_start(out=ident_t[:], in_=ident_d[:])

            for l, plan in enumerate(plans):
                outw = OUTW[l]
                table, dest = tables[l], outs[l]

                # resident per-layer constants
                idx_t = cpool.tile(list(idx_d[l].shape), idx_d[l].dtype, tag=f"idx{l}")
                nc.sync.dma_start(out=idx_t[:], in_=idx_d[l][:])
                edst_t = cpool.tile([P, plan.n_cols_pad], f32, tag=f"edst{l}")
                nc.sync.dma_start(out=edst_t[:], in_=edst_d[l][:])
                invd_t = cpool.tile([P, plan.n_cols_pad], f32, tag=f"invd{l}")
                nc.sync.dma_start(out=invd_t[:], in_=invd_d[l][:])
                ws_t, wn_t = [], []
                for k in range(2):
                    w = cpool.tile([P, outw], f32, tag=f"ws{l}_{k}")
                    nc.sync.dma_start(out=w[:], in_=w_d[l][0][k * P : (k + 1) * P, :])
                    ws_t.append(w)
                    w = cpool.tile([P, outw], f32, tag=f"wn{l}_{k}")
                    nc.sync.dma_start(out=w[:], in_=w_d[l][1][k * P : (k + 1) * P, :])
                    wn_t.append(w)
                bias_t = cpool.tile([P, outw], f32, tag=f"bias{l}")
                nc.sync.dma_start(out=bias_t[:], in_=w_d[l][2][:])

                # gather calls (Tile pipelines them via the msgs pool)
                call_tiles = []
                if USE_INDIRECT:
                    for col in range(plan.n_cols_pad):
                        mt = mpool.tile([P, FEAT], f32, tag="msgs")
                        nc.gpsimd.indirect_dma_start(
                            out=mt[:],
                            out_offset=None,
                            in_=table[:],
                            in_offset=bass.IndirectOffsetOnAxis(
                                ap=idx_t[:, col : col + 1], axis=0
                            ),
                        )
                        call_tiles.append(mt)

                    def chunk_slice(col, f0, f1):
                        return call_tiles[col][:, f0:f1]
                else:
                    for k in range(plan.n_calls):
                        mt = mpool.tile([P, CPC * FEAT], f32, tag="msgs")
                        base = int(plan.call_base[0][k])
                        hi = min(base + WINDOW, table.shape[0])
                        nc.gpsimd.dma_gather(
                            out_ap=mt[:].rearrange("p (g d) -> p g d", g=CPC),
                            in_ap=table[base:hi, :],
                            idxs_ap=idx_t[:, k * (NI // 16) : (k + 1) * (NI // 16)],
                            num_idxs=NI,
                            num_idxs_reg=NI,
                            elem_size=FEAT,
                        )
                        call_tiles.append(mt)

                    def chunk_slice(col, f0, f1):
                        k, j = divmod(col, CPC)
                        return call_tiles[k][:, j * FEAT + f0 : j * FEAT + f1]

                for t in range(plan.n_tiles):
                    # h_dstT via self chunk against identity
                    ph0 = pa.tile([P, P], f32, tag="ph0")
                    ph1 = pa.tile([P, P], f32, tag="ph1")
                    nc.tensor.matmul(ph0[:], lhsT=chunk_slice(plan.col_self[t], 0, P),
                                     rhs=ident_t[:], start=True, stop=True)
                    nc.tensor.matmul(ph1[:], lhsT=chunk_slice(plan.col_self[t], P, 2 * P),
                                     rhs=ident_t[:], start=True, stop=True)
                    # meanT via S' chunks
                    pa0 = pa.tile([P, P], f32, tag="pa0")
                    pa1 = pa.tile([P, P], f32, tag="pa1")
                    cols = plan.col_edges[t]
                    for i, col in enumerate(cols):
                        S = spool.tile([P, P], f32, tag="S")
                        nc.vector.tensor_scalar(
                            out=S[:],
                            in0=iota_t[:],
                            scalar1=edst_t[:, col : col + 1],
                            scalar2=invd_t[:, col : col + 1],
                            op0=mybir.AluOpType.is_equal,
                            op1=mybir.AluOpType.mult,
                        )
                        nc.tensor.matmul(pa0[:], lhsT=chunk_slice(col, 0, P), rhs=S[:],
                                         start=(i == 0), stop=(i == len(cols) - 1))
                        nc.tensor.matmul(pa1[:], lhsT=chunk_slice(col, P, 2 * P), rhs=S[:],
                                         start=(i == 0), stop=(i == len(cols) - 1))
                    # PSUM -> SBUF
                    a0 = apool.tile([P, P], f32, tag="a0")
                    nc.vector.tensor_copy(out=a0[:], in_=pa0[:])
                    a1 = apool.tile([P, P], f32, tag="a1")
                    nc.vector.tensor_copy(out=a1[:], in_=pa1[:])
                    h0 = apool.tile([P, P], f32, tag="h0")
                    nc.vector.tensor_copy(out=h0[:], in_=ph0[:])
                    h1 = apool.tile([P, P], f32, tag="h1")
                    nc.vector.tensor_copy(out=h1[:], in_=ph1[:])
                    # Y = meanT.T @ Wn + h_dstT.T @ Ws
                    y = pypool.tile([P, outw], f32, tag="y")
                    nc.tensor.matmul(y[:], lhsT=a0[:], rhs=wn_t[0][:],
                                     start=True, stop=False)
                    nc.tensor.matmul(y[:], lhsT=a1[:], rhs=wn_t[1][:],
                                     start=False, stop=False)
                    nc.tensor.matmul(y[:], lhsT=h0[:], rhs=ws_t[0][:],
                                     start=False, stop=False)
                    nc.tensor.matmul(y[:], lhsT=h1[:], rhs=ws_t[1][:],
                                     start=False, stop=True)
                    o = opool.tile([P, outw], f32, tag="o")
                    nc.vector.tensor_tensor(out=o[:], in0=y[:], in1=bias_t[:],
                                            op=mybir.AluOpType.add)
                    if l < 2:
                        o2 = opool.tile([P, outw], f32, tag="o2")
                        nc.scalar.activation(
                            out=o2[:], in_=o[:],
                            func=mybir.ActivationFunctionType.Relu,
                        )
                        nc.sync.dma_start(
                            out=dest[t * P : (t + 1) * P, :], in_=o2[:]
                        )
                    else:
                        nc.sync.dma_start(
                            out=dest[:], in_=o[0:SEEDS_PER_CORE, :]
                        )
                if l < 2:
                    tc.strict_bb_all_engine_barrier()

    nc.compile()

    in_maps = []
    for c in range(NCORES):
        m = dict(
            xc=meta["xcs"][c],
            iota=np.broadcast_to(
                np.arange(P, dtype=np.float32)[None, :], (P, P)
            ).copy(),
            ident=np.eye(P, dtype=np.float32),
        )
        for l, plan in enumerate(plans):
            if USE_INDIRECT:
                m[f"gidx{l}"] = plan.gidx[c].astype(np.int32)
            else:
                rel = plan.gidx[c] - np.repeat(plan.call_base[c], CPC)[None, :]
                m[f"gidx{l}"] = _wrap_idx16(rel)
            m[f"edst{l}"] = plan.edst[c]
            m[f"invd{l}"] = plan.invd[c]
            ws, wn, b = meta["weights"][l]
            m[f"ws{l}"] = np.ascontiguousarray(ws)
            m[f"wn{l}"] = np.ascontiguousarray(wn)
            m[f"bias{l}"] = np.broadcast_to(b[None, :], (P, OUTW[l])).copy()
        in_maps.append(m)

    res = run_bass_kernel_spmd(
        nc, in_maps, core_ids=list(range(NCORES)), trace=trace
    )
    return [res.results[c]["out"] for c in range(NCORES)], res


def kernel(**inputs) -> np.ndarray:
    meta = build_host(inputs)
    outs, _ = run_device(meta)
    return np.concatenate(outs, axis=0)


# revision 12
# speedup vs baseline: 1.2121x; 1.2121x over previous
"""DistSAGE 3-layer GraphSAGE forward on 8 TRN2 NeuronCores (Bass/Tile).

Strategy (graph/data parallel, per the DistSAGE recipe):
  - Partition the 512 seed nodes across 8 cores (64 each); build per-core
    dependency-driven blocks on the host (standard DGL block construction,
    pure index math): layer-2 dsts = seeds, layer-1 dsts = seeds + their
    layer-2 sources, layer-0 dsts = layer-1 dsts + their layer-1 sources.
    No inter-core communication; weights replicated.
  - Row-shard the feature table: each core gets a compact bf16 table with
    only the x rows its block touches, laid out in per-dst-tile "bands" so
    each dma_gather call addresses rows with int16 indices relative to a
    32768-row window (the gather ucode's index width).
  - Per 128-dst tile on device: dma_gather edge messages (128 edges/chunk,
    1024/call), accumulate meanT[f,d] += msgs.T @ S' on the TensorEngine
    in PSUM, where S'[e,d] = (edst[e]==d)/deg[d] is a host-precomputed
    bf16 selection matrix DMA'd on the (otherwise idle) HWDGE line.  A
    per-tile "self chunk" whose S' is the identity yields h_dstT the same
    way (doubling as the transpose).  Then Y[d,:] = meanT.T@W_neigh +
    h_dstT.T@W_self (4 PSUM-accumulated bf16 matmuls), + bias, ReLU, DMA
    the bf16 tile to DRAM for the next layer's gather.
"""

import heapq

import numpy as np

P = 128
NCORES = 8
NUM_DST = (61952, 5632, 512)
FEAT = 256
OUTW = (256, 256, 19)
SEEDS_PER_CORE = NUM_DST[2] // NCORES  # 64
WINDOW = 32768
NI = 1024  # indices per dma_gather call (ring-safe max)
CPC = NI // P  # chunks per call = 8


def _bf16():
    import ml_dtypes

    return ml_dtypes.bfloat16


# ---------------------------------------------------------------------------
# Host-side block construction
# ---------------------------------------------------------------------------


def _balance(ids, deg, n_buckets):
    """LPT bin-packing: reorder ids so consecutive 128-groups have ~equal
    total degree. len(ids) must be n_buckets * 128."""
    if n_buckets <= 1:
        return ids
    order = np.argsort(-deg[ids], kind="stable")
    heap = [(0.0, b, 0) for b in range(n_buckets)]  # (load, bucket, count)
    heapq.heapify(heap)
    buckets = [[] for _ in range(n_buckets)]
    for i in order:
        load, b, cnt = heapq.heappop(heap)
        buckets[b].append(ids[i])
        cnt += 1
        if cnt < P:
            heapq.heappush(heap, (load + deg[ids[i]], b, cnt))
    return np.concatenate([np.asarray(b, dtype=ids.dtype) for b in buckets])


def _block_for_core(c, esrc0, edst0, esrc1, edst1, esrc2, edst2, deg0, deg1, deg2):
    seeds = np.arange(c * SEEDS_PER_CORE, (c + 1) * SEEDS_PER_CORE, dtype=np.int64)

    # layer 2: seeds are the dsts
    sel2 = (edst2 >= seeds[0]) & (edst2 < seeds[0] + SEEDS_PER_CORE)
    es2, ed2g = esrc2[sel2], edst2[sel2]
    l1_extra = np.setdiff1d(np.unique(es2), seeds)
    # balance layer-1 tiles by deg1 (full 128-groups of the extras segment)
    nfull = ((SEEDS_PER_CORE + len(l1_extra)) // P) * P - SEEDS_PER_CORE
    nfull = (len(l1_extra) // P) * P
    l1_extra = np.concatenate(
        [_balance(l1_extra[:nfull], deg1, nfull // P), l1_extra[nfull:]]
    ) if nfull >= P else l1_extra
    l1_out = np.concatenate([seeds, l1_extra])
    n1 = len(l1_out)

    # layer 1: dsts = l1_out
    pos1 = np.full(NUM_DST[1], -1, np.int32)
    pos1[l1_out] = np.arange(n1, dtype=np.int32)
    sel1 = pos1[edst1] >= 0
    es1, ed1g = esrc1[sel1], edst1[sel1]
    ed1 = pos1[ed1g].astype(np.int64)
    inv1 = (1.0 / np.maximum(deg1[ed1g], 1.0)).astype(np.float32)
    l0_extra = np.setdiff1d(np.unique(es1), l1_out)
    nfull = (len(l0_extra) // P) * P
    l0_extra = np.concatenate(
        [_balance(l0_extra[:nfull], deg0, nfull // P), l0_extra[nfull:]]
    ) if nfull >= P else l0_extra
    l0_out = np.concatenate([l1_out, l0_extra])
    n0 = len(l0_out)

    # layer 0: dsts = l0_out
    pos0 = np.full(NUM_DST[0], -1, np.int32)
    pos0[l0_out] = np.arange(n0, dtype=np.int32)
    sel0 = pos0[edst0] >= 0
    es0, ed0g = esrc0[sel0], edst0[sel0]
    ed0 = pos0[ed0g].astype(np.int64)
    inv0 = (1.0 / np.maximum(deg0[ed0g], 1.0)).astype(np.float32)

    ed2 = pos1[ed2g].astype(np.int64)  # seeds occupy slots 0..63
    inv2 = (1.0 / np.maximum(deg2[ed2g], 1.0)).astype(np.float32)
    es2l = pos1[es2].astype(np.int64)  # h2 local rows
    es1l = pos0[es1].astype(np.int64)  # h1 local rows

    return dict(
        l1_out=l1_out, l0_out=l0_out, n1=n1, n0=n0,
        e0=(es0.astype(np.int64), ed0, inv0),
        e1=(es1l, ed1, inv1),
        e2=(es2l, ed2, inv2),
    )


def _group_edges_by_tile(es, ed, inv, n_tiles):
    """Sort edges by (dst tile, src); return per-tile (src, dst%128, inv)."""
    tile = ed // P
    order = np.lexsort((es, tile))
    es, ed, inv, tile = es[order], ed[order], inv[order], tile[order]
    starts = np.searchsorted(tile, np.arange(n_tiles))
    ends = np.searchsorted(tile, np.arange(n_tiles) + 1)
    return [
        (es[s:e], ed[s:e] - t * P, inv[s:e])
        for t, (s, e) in enumerate(zip(starts, ends))
    ]


class LayerPlan:
    def __init__(self, n_tiles, chunks_per_tile):
        self.n_tiles = n_tiles
        self.chunks_per_tile = chunks_per_tile
        self.col_self = []
        self.col_edges = []
        col = 0
        for t in range(n_tiles):
            self.col_self.append(col)
            col += 1
            self.col_edges.append(list(range(col, col + chunks_per_tile[t])))
            col += chunks_per_tile[t]
        self.n_cols = col
        self.n_calls = -(-col // CPC)
        self.n_cols_pad = self.n_calls * CPC
        self.gidx = []  # [NCORES][128, n_cols_pad] int64 table rows
        self.edst = []  # [NCORES][128, n_cols_pad] f32, -1 = padding
        self.invd = []  # [NCORES][128, n_cols_pad] f32
        self.call_base = None  # [n_calls] int64, uniform across cores


def _plan_layer(per_core_tiles, n_tiles):
    chunks = [
        max(1, max(-(-len(per_core_tiles[c][t][0]) // P) for c in range(NCORES)))
        for t in range(n_tiles)
    ]
    return LayerPlan(n_tiles, chunks)


def _fill_plan(plan, per_core_tiles, self_rows, pad_row):
    """self_rows[c][t]: int64[128] table rows for the tile's h_dst.
    pad_row[c][t]: table row used for padded slots (must lie in the band)."""
    for c in range(NCORES):
        gidx = np.zeros((P, plan.n_cols_pad), np.int64)
        edst = np.full((P, plan.n_cols_pad), -1.0, np.float32)
        invd = np.zeros((P, plan.n_cols_pad), np.float32)
        for t in range(plan.n_tiles):
            gidx[:, plan.col_self[t]] = self_rows[c][t]
            es, ed, inv = per_core_tiles[c][t]
            n = len(es)
            cols = plan.col_edges[t]
            bi = np.full(len(cols) * P, pad_row[c][t], np.int64)
            bd = np.full(len(cols) * P, -1.0, np.float32)
            bv = np.zeros(len(cols) * P, np.float32)
            bi[:n], bd[:n], bv[:n] = es, ed, inv
            for j, col in enumerate(cols):
                gidx[:, col] = bi[j * P : (j + 1) * P]
                edst[:, col] = bd[j * P : (j + 1) * P]
                invd[:, col] = bv[j * P : (j + 1) * P]
        if plan.n_cols_pad > plan.n_cols:
            gidx[:, plan.n_cols :] = pad_row[c][plan.n_tiles - 1]
        plan.gidx.append(gidx)
        plan.edst.append(edst)
        plan.invd.append(invd)


def _sprime(plan, c):
    """Host-precomputed selection matrices, bf16: [128, n_cols_pad*128].
    Edge col: S'[e, d] = (edst==d)*invd.  Self col: identity."""
    bf16 = _bf16()
    ed = plan.edst[c][:, :, None]
    iv = plan.invd[c][:, :, None]
    S = (ed == np.arange(P, dtype=np.float32)[None, None, :]) * iv
    S = S.astype(bf16)
    eye = np.eye(P, dtype=bf16)
    for t in range(plan.n_tiles):
        S[:, plan.col_self[t], :] = eye
    return np.ascontiguousarray(S.reshape(P, plan.n_cols_pad * P))


def build_host(inputs):
    esrc0 = np.asarray(inputs["esrc0"]).astype(np.int64)
    edst0 = np.asarray(inputs["edst0"]).astype(np.int64)
    esrc1 = np.asarray(inputs["esrc1"]).astype(np.int64)
    edst1 = np.asarray(inputs["edst1"]).astype(np.int64)
    esrc2 = np.asarray(inputs["esrc2"]).astype(np.int64)
    edst2 = np.asarray(inputs["edst2"]).astype(np.int64)
    x = np.asarray(inputs["x"], dtype=np.float32)

    deg0 = np.bincount(edst0, minlength=NUM_DST[0]).astype(np.float32)
    deg1 = np.bincount(edst1, minlength=NUM_DST[1]).astype(np.float32)
    deg2 = np.bincount(edst2, minlength=NUM_DST[2]).astype(np.float32)

    blocks = [
        _block_for_core(c, esrc0, edst0, esrc1, edst1, esrc2, edst2, deg0, deg1, deg2)
        for c in range(NCORES)
    ]

    n0_pad = max(-(-b["n0"] // P) for b in blocks) * P
    n1_pad = max(-(-b["n1"] // P) for b in blocks) * P
    T0, T1, T2 = n0_pad // P, n1_pad // P, 1

    tiles0 = [_group_edges_by_tile(*b["e0"], T0) for b in blocks]
    tiles1 = [_group_edges_by_tile(*b["e1"], T1) for b in blocks]
    tiles2 = [_group_edges_by_tile(*b["e2"], T2) for b in blocks]

    plan0 = _plan_layer(tiles0, T0)
    plan1 = _plan_layer(tiles1, T1)
    plan2 = _plan_layer(tiles2, T2)

    # layer-0 self rows = the tiles' global dst ids (padded slots -> first
    # dst of the tile; masked downstream since nothing references them)
    l0_padded = []
    for b in blocks:
        v = np.zeros(T0 * P, np.int64)
        v[: b["n0"]] = b["l0_out"]
        v[b["n0"] :] = b["l0_out"][0]
        l0_padded.append(v)
    self0 = [
        [l0_padded[c][t * P : (t + 1) * P] for t in range(T0)] for c in range(NCORES)
    ]
    pad0 = [[l0_padded[c][t * P] for t in range(T0)] for c in range(NCORES)]
    _fill_plan(plan0, tiles0, self0, pad0)

    ar = np.arange(P, dtype=np.int64)
    selfL = lambda T: [[t * P + ar for t in range(T)] for _ in range(NCORES)]
    padL = lambda T: [[t * P for t in range(T)] for _ in range(NCORES)]
    _fill_plan(plan1, tiles1, selfL(T1), padL(T1))
    _fill_plan(plan2, tiles2, selfL(T2), padL(T2))

    # ---- layer 0: banded compact x table per core + index remap ----
    # Band offsets must be uniform across cores (window bases are baked into
    # the SPMD-shared graph): pad each band to the max size over cores.
    bands = []
    for c in range(NCORES):
        bands.append(
            [
                np.unique(plan0.gidx[c][:, [plan0.col_self[t]] + plan0.col_edges[t]])
                for t in range(T0)
            ]
        )
    band_size = np.array(
        [max(len(bands[c][t]) for c in range(NCORES)) for t in range(T0)], np.int64
    )
    band_start = np.concatenate([[0], np.cumsum(band_size)])

    for c in range(NCORES):
        gidx, new = plan0.gidx[c], np.zeros_like(plan0.gidx[c])
        for t in range(T0):
            band = bands[c][t]
            cols = [plan0.col_self[t]] + plan0.col_edges[t]
            sl = gidx[:, cols]
            loc = np.searchsorted(band, sl)
            assert (band[loc] == sl).all()
            new[:, cols] = band_start[t] + loc
        if plan0.n_cols_pad > plan0.n_cols:
            band = bands[c][T0 - 1]
            sl = gidx[:, plan0.n_cols :]
            new[:, plan0.n_cols :] = band_start[T0 - 1] + np.searchsorted(band, sl)
        plan0.gidx[c] = new

    col_tile = np.zeros(plan0.n_cols_pad, np.int64)
    for t in range(T0):
        for col in [plan0.col_self[t]] + plan0.col_edges[t]:
            col_tile[col] = t
    col_tile[plan0.n_cols :] = T0 - 1
    plan0.call_base = np.array(
        [band_start[col_tile[k * CPC]] for k in range(plan0.n_calls)], np.int64
    )
    for c in range(NCORES):
        for k in range(plan0.n_calls):
            sl = plan0.gidx[c][:, k * CPC : (k + 1) * CPC]
            assert sl.min() >= plan0.call_base[k], (c, k)
            assert sl.max() < plan0.call_base[k] + WINDOW, (c, k)
    plan1.call_base = np.zeros(plan1.n_calls, np.int64)
    plan2.call_base = np.zeros(plan2.n_calls, np.int64)
    assert n0_pad <= WINDOW and n1_pad <= WINDOW

    bf16 = _bf16()
    x16 = x.astype(bf16)
    xc_len_pad = -(-int(band_start[T0]) // P) * P
    xcs = []
    for c in range(NCORES):
        t = np.zeros((xc_len_pad, FEAT), bf16)
        for tt in range(T0):
            b = bands[c][tt]
            t[band_start[tt] : band_start[tt] + len(b)] = x16[b]
        xcs.append(t)

    return dict(
        plans=(plan0, plan1, plan2),
        T=(T0, T1, T2),
        n0_pad=n0_pad,
        n1_pad=n1_pad,
        xc_len_pad=xc_len_pad,
        xcs=xcs,
        blocks=blocks,
        weights=tuple(
            (
                np.asarray(inputs[f"W_self{l}"], np.float32),
                np.asarray(inputs[f"W_neigh{l}"], np.float32),
                np.asarray(inputs[f"b{l}"], np.float32),
            )
            for l in range(3)
        ),
    )


# ---------------------------------------------------------------------------
# Numpy simulation of the device kernel (validation aid; fp32 stand-in)
# ---------------------------------------------------------------------------


def simulate_core(meta, c):
    table = meta["xcs"][c].astype(np.float32)
    for l, plan in enumerate(meta["plans"]):
        ws, wn, b = meta["weights"][l]
        out = np.zeros((plan.n_tiles * P, OUTW[l]), np.float32)
        for t in range(plan.n_tiles):
            hd = table[plan.gidx[c][:, plan.col_self[t]]]
            aggT = np.zeros((FEAT, P), np.float32)
            for col in plan.col_edges[t]:
                msgs = table[plan.gidx[c][:, col]]
                ed = plan.edst[c][:, col]
                iv = plan.invd[c][:, col]
                S = (ed[:, None] == np.arange(P)[None, :]) * iv[:, None]
                aggT += msgs.T @ S.astype(np.float32)
            y = hd @ ws + aggT.T @ wn + b
            if l < 2:
                y = np.maximum(y, 0.0)
            out[t * P : (t + 1) * P] = y
        table = out
    return table[:SEEDS_PER_CORE]


# ---------------------------------------------------------------------------
# Device kernel
# ---------------------------------------------------------------------------


def _wrap_idx16(gidx_rel):
    """[128, n_cols_pad] relative rows -> dma_gather idx layout
    [128, n_calls*NI/16] int16 (16-partition wrap, replicated x8)."""
    n_calls = gidx_rel.shape[1] // CPC
    out = np.zeros((P, n_calls * NI // 16), np.int16)
    for k in range(n_calls):
        flat = gidx_rel[:, k * CPC : (k + 1) * CPC].T.reshape(-1)  # i = j*128+p
        w = flat.reshape(NI // 16, 16).T.astype(np.int16)
        out[:16, k * (NI // 16) : (k + 1) * (NI // 16)] = w
    for rep in range(1, 8):
        out[rep * 16 : (rep + 1) * 16] = out[:16]
    return out


def run_device(meta, trace=False):
    import concourse.bacc as bacc
    import concourse.tile as tile
    import concourse.mybir as mybir
    from concourse.bass_utils import run_bass_kernel_spmd

    plans = meta["plans"]
    f32 = mybir.dt.float32
    b16 = mybir.dt.bfloat16

    nc = bacc.Bacc("TRN2", target_bir_lowering=False, debug=False, num_devices=NCORES)

    xc = nc.dram_tensor("xc", [meta["xc_len_pad"], FEAT], b16, kind="ExternalInput")
    h1buf = nc.dram_tensor("h1buf", [meta["n0_pad"], FEAT], b16)
    h2buf = nc.dram_tensor("h2buf", [meta["n1_pad"], FEAT], b16)
    out_d = nc.dram_tensor("out", [SEEDS_PER_CORE, OUTW[2]], f32, kind="ExternalOutput")

    idx_d, sp_d, w_d = [], [], []
    for l, plan in enumerate(plans):
        idx_d.append(
            nc.dram_tensor(f"gidx{l}", [P, plan.n_calls * NI // 16], mybir.dt.int16,
                           kind="ExternalInput")
        )
        sp_d.append(
            nc.dram_tensor(f"sp{l}", [P, plan.n_cols_pad * P], b16,
                           kind="ExternalInput")
        )
        w_d.append(
            (
                nc.dram_tensor(f"ws{l}", [FEAT, OUTW[l]], b16, kind="ExternalInput"),
                nc.dram_tensor(f"wn{l}", [FEAT, OUTW[l]], b16, kind="ExternalInput"),
                nc.dram_tensor(f"bias{l}", [P, OUTW[l]], f32, kind="ExternalInput"),
            )
        )

    tables = [xc, h1buf, h2buf]
    dests = [h1buf, h2buf, out_d]

    with tile.TileContext(nc) as tc:
        with (
            tc.tile_pool(name="const", bufs=1) as cpool,
            tc.tile_pool(name="msgs", bufs=4) as mpool,
            tc.tile_pool(name="sel", bufs=4) as spool,
            tc.tile_pool(name="acc", bufs=2) as apool,
            tc.tile_pool(name="outp", bufs=3) as opool,
            tc.tile_pool(name="pagg", bufs=1, space="PSUM") as pa,
            tc.tile_pool(name="py", bufs=2, space="PSUM") as pypool,
        ):
            for l, plan in enumerate(plans):
                outw = OUTW[l]
                table, dest = tables[l], dests[l]

                idx_t = cpool.tile(list(idx_d[l].shape), mybir.dt.int16, tag=f"idx{l}")
                nc.sync.dma_start(out=idx_t[:], in_=idx_d[l][:])
                ws_t, wn_t = [], []
                for k in range(2):
                    w = cpool.tile([P, outw], b16, tag=f"ws{l}_{k}")
                    nc.sync.dma_start(out=w[:], in_=w_d[l][0][k * P : (k + 1) * P, :])
                    ws_t.append(w)
                    w = cpool.tile([P, outw], b16, tag=f"wn{l}_{k}")
                    nc.sync.dma_start(out=w[:], in_=w_d[l][1][k * P : (k + 1) * P, :])
                    wn_t.append(w)
                bias_t = cpool.tile([P, outw], f32, tag=f"bias{l}")
                nc.sync.dma_start(out=bias_t[:], in_=w_d[l][2][:])

                # gather calls + S' slab loads (Tile pipelines via the pools)
                call_tiles, sp_tiles = [], []
                for k in range(plan.n_calls):
                    mt = mpool.tile([P, CPC * FEAT], b16, tag="msgs")
                    base = int(plan.call_base[k])
                    hi = min(base + WINDOW, table.shape[0])
                    nc.gpsimd.dma_gather(
                        out_ap=mt[:].rearrange("p (g d) -> p g d", g=CPC),
                        in_ap=table[base:hi, :],
                        idxs_ap=idx_t[:, k * (NI // 16) : (k + 1) * (NI // 16)],
                        num_idxs=NI,
                        num_idxs_reg=NI,
                        elem_size=FEAT,
                    )
                    call_tiles.append(mt)
                    st = spool.tile([P, CPC * P], b16, tag="sp")
                    nc.sync.dma_start(
                        out=st[:], in_=sp_d[l][:, k * CPC * P : (k + 1) * CPC * P]
                    )
                    sp_tiles.append(st)

                def msg_slice(col, f0, f1):
                    k, j = divmod(col, CPC)
                    return call_tiles[k][:, j * FEAT + f0 : j * FEAT + f1]

                def sp_slice(col):
                    k, j = divmod(col, CPC)
                    return sp_tiles[k][:, j * P : (j + 1) * P]

                for t in range(plan.n_tiles):
                    cs = plan.col_self[t]
                    ph0 = pa.tile([P, P], f32, tag="ph0")
                    ph1 = pa.tile([P, P], f32, tag="ph1")
                    nc.tensor.matmul(ph0[:], lhsT=msg_slice(cs, 0, P),
                                     rhs=sp_slice(cs), start=True, stop=True)
                    nc.tensor.matmul(ph1[:], lhsT=msg_slice(cs, P, 2 * P),
                                     rhs=sp_slice(cs), start=True, stop=True)
                    pa0 = pa.tile([P, P], f32, tag="pa0")
                    pa1 = pa.tile([P, P], f32, tag="pa1")
                    cols = plan.col_edges[t]
                    for i, col in enumerate(cols):
                        st, sp = (i == 0), (i == len(cols) - 1)
                        nc.tensor.matmul(pa0[:], lhsT=msg_slice(col, 0, P),
                                         rhs=sp_slice(col), start=st, stop=sp)
                        nc.tensor.matmul(pa1[:], lhsT=msg_slice(col, P, 2 * P),
                                         rhs=sp_slice(col), start=st, stop=sp)
                    a0 = apool.tile([P, P], b16, tag="a0")
                    nc.vector.tensor_copy(out=a0[:], in_=pa0[:])
                    a1 = apool.tile([P, P], b16, tag="a1")
                    nc.vector.tensor_copy(out=a1[:], in_=pa1[:])
                    h0 = apool.tile([P, P], b16, tag="h0")
                    nc.vector.tensor_copy(out=h0[:], in_=ph0[:])
                    h1 = apool.tile([P, P], b16, tag="h1")
                    nc.vector.tensor_copy(out=h1[:], in_=ph1[:])
                    y = pypool.tile([P, outw], f32, tag="y")
                    nc.tensor.matmul(y[:], lhsT=a0[:], rhs=wn_t[0][:],
                                     start=True, stop=False)
                    nc.tensor.matmul(y[:], lhsT=a1[:], rhs=wn_t[1][:],
                                     start=False, stop=False)
                    nc.tensor.matmul(y[:], lhsT=h0[:], rhs=ws_t[0][:],
                                     start=False, stop=False)
                    nc.tensor.matmul(y[:], lhsT=h1[:], rhs=ws_t[1][:],
                                     start=False, stop=True)
                    if l < 2:
                        o = opool.tile([P, outw], f32, tag="o")
                        nc.vector.tensor_tensor(out=o[:], in0=y[:], in1=bias_t[:],
                                                op=mybir.AluOpType.add)
                        o2 = opool.tile([P, outw], b16, tag="o2")
                        nc.scalar.activation(
                            out=o2[:], in_=o[:],
                            func=mybir.ActivationFunctionType.Relu,
                        )
                        nc.sync.dma_start(out=dest[t * P : (t + 1) * P, :], in_=o2[:])
                    else:
                        o = opool.tile([P, outw], f32, tag="o")
                        nc.vector.tensor_tensor(out=o[:], in0=y[:], in1=bias_t[:],
                                                op=mybir.AluOpType.add)
                        nc.sync.dma_start(out=dest[:], in_=o[0:SEEDS_PER_CORE, :])
                if l < 2:
                    tc.strict_bb_all_engine_barrier()

    nc.compile()

    in_maps = []
    bf16 = _bf16()
    for c in range(NCORES):
        m = dict(xc=meta["xcs"][c])
        for l, plan in enumerate(plans):
            rel = plan.gidx[c] - np.repeat(plan.call_base, CPC)[None, :]
            m[f"gidx{l}"] = _wrap_idx16(rel)
            m[f"sp{l}"] = _sprime(plan, c)
            ws, wn, b = meta["weights"][l]
            m[f"ws{l}"] = np.ascontiguousarray(ws.astype(bf16))
            m[f"wn{l}"] = np.ascontiguousarray(wn.astype(bf16))
            m[f"bias{l}"] = np.broadcast_to(b[None, :], (P, OUTW[l])).copy()
        in_maps.append(m)

    res = run_bass_kernel_spmd(
        nc, in_maps, core_ids=list(range(NCORES)), trace=trace
    )
    return [res.results[c]["out"] for c in range(NCORES)], res


def kernel(**inputs) -> np.ndarray:
    meta = build_host(inputs)
    outs, _ = run_device(meta)
    return np.concatenate(outs, axis=0)
